# revision 1
# baseline (speedup 1.0000x reference)
"""Trainium2 Bass kernel for GroundTruthBasedPriorNetwork.

Per-node tiny MLP over a banded DAG, batched over 131072 samples:
    x[b, n, p]  = gt_labels[b, parent_idx[n, p]]          (N=64 nodes, P=8)
    h[b, n, :]  = tanh(W1[n] @ x[b, n, :] + b1[n])        (HID=16)
    mus[b, n]   = W2[n] . h[b, n, :] + b2[n]
    logvars     = zeros

Strategy: pure data parallel over 8 NeuronCores (batch split 8x16384).
The parent gather is folded on the host into a dense (64 x 1024) matrix
W1_full with W1_full[j, 16n+h] = sum_p [parent_idx[n,p]==j] W1[n,h,p],
augmented with a bias row (row 64) that multiplies a ones-row appended to
the host-transposed input XT (65 x 16384).  On-device per 256-column
group: 8 fp32r matmuls -> PSUM (128x2048) = h_pre^T, one Tanh activation
instruction (ScalarE is the bottleneck engine), 8 accumulating matmuls
against a block-structured W2T (128x512) -> mus^T (64x256), then a DVE
tensor_scalar add of b2 that also evacuates PSUM->SBUF.  Output stays
node-major (64 x 16384) per core and is un-transposed on the host.
"""

import os

import numpy as np

NUM_NODES = 64
MAX_P = 8
HID = 16
HFULL = NUM_NODES * HID  # 1024
BATCH = 131072
NCORES = 8
BC = BATCH // NCORES  # 16384 per core
GROUP = 256  # batch columns per group
NG = BC // GROUP  # 64 groups
OUT_CHUNK = 16  # groups per output DMA chunk

_COMPILED = {}


def _build_weights(W1, b1, W2, b2, parent_idx):
    """Host-side preprocessing of the tiny per-node weights."""
    W1 = np.asarray(W1, np.float32)
    b1 = np.asarray(b1, np.float32)
    W2 = np.asarray(W2, np.float32)
    b2 = np.asarray(b2, np.float32)
    parent_idx = np.asarray(parent_idx)

    # W1_full[j, 16n+h] = sum_p [parent_idx[n,p]==j] * W1[n,h,p]
    w1_full = np.zeros((NUM_NODES, HFULL), np.float32)
    for n in range(NUM_NODES):
        for p in range(MAX_P):
            j = int(parent_idx[n, p])
            w1_full[j, 16 * n : 16 * n + 16] += W1[n, :, p]
    w1_aug = np.concatenate([w1_full, b1.reshape(1, HFULL)], axis=0)  # (65, 1024)

    # W2T[p, 64t+n] = W2[n, hf%16] where hf = 128t+p and n == hf//16, else 0
    w2t = np.zeros((128, 8 * NUM_NODES), np.float32)
    for t in range(8):
        for p in range(128):
            hf = 128 * t + p
            n = hf // HID
            w2t[p, NUM_NODES * t + n] = W2[n, hf % HID]

    wpack = np.zeros((128, HFULL + 8 * NUM_NODES + 1), np.float32)
    wpack[: NUM_NODES + 1, :HFULL] = w1_aug
    wpack[:, HFULL : HFULL + 8 * NUM_NODES] = w2t
    wpack[:NUM_NODES, HFULL + 8 * NUM_NODES] = b2
    return np.ascontiguousarray(wpack)


def _build_nc():
    import concourse.bacc as bacc
    import concourse.mybir as mybir
    import concourse.tile as tile
    from contextlib import ExitStack

    f32 = mybir.dt.float32
    f32r = mybir.dt.float32r

    nc = bacc.Bacc("TRN2", target_bir_lowering=False, debug=False,
                   num_devices=NCORES)

    # All per-node weights packed into one (128, 1537) tensor / one DMA:
    # cols [0,1024) = W1aug (rows 0-64), [1024,1536) = W2T, col 1536 = b2.
    CW = HFULL + 8 * NUM_NODES + 1
    xt_d = nc.dram_tensor("xt", [NUM_NODES + 1, BC], f32r, kind="ExternalInput")
    wpack_d = nc.dram_tensor("wpack", [128, CW], f32r, kind="ExternalInput")
    out_d = nc.dram_tensor("out", [NUM_NODES, BC], f32, kind="ExternalOutput")

    NXT = 4  # number of xt sbuf tiles / input DMA chunks
    XTW = BC // NXT  # 4096 columns each
    NOC = NG // OUT_CHUNK  # 4 output chunks
    OCW = OUT_CHUNK * GROUP  # 4096

    with tile.TileContext(nc) as tc, ExitStack() as ctx:
        consts = ctx.enter_context(tc.tile_pool(name="consts", bufs=1))
        xt_pool = ctx.enter_context(tc.tile_pool(name="xt", bufs=1))
        out_pool = ctx.enter_context(tc.tile_pool(name="out", bufs=1))
        h_pool = ctx.enter_context(tc.tile_pool(name="h", bufs=2))
        l1_pool = ctx.enter_context(tc.tile_pool(name="l1", bufs=1, space="PSUM"))
        mus_pool = ctx.enter_context(tc.tile_pool(name="mus", bufs=2, space="PSUM"))

        wpack_sb = consts.tile([128, CW], f32r, tag="wpack")
        nc.sync.dma_start(wpack_sb[:], wpack_d.ap())
        w1_sb = wpack_sb[: NUM_NODES + 1, :HFULL]
        w2t_sb = wpack_sb[:, HFULL : HFULL + 8 * NUM_NODES]
        b2_sb = wpack_sb[:NUM_NODES, HFULL + 8 * NUM_NODES :].bitcast(f32)

        xt_tiles = []
        for k in range(NXT):
            xt_sb = xt_pool.tile(
                [NUM_NODES + 1, XTW], f32r, tag=f"xt{k}", name=f"xt_sb{k}"
            )
            xt_tiles.append(xt_sb)

        # Load the first batch chunk, then fence: the barrier absorbs the
        # const + first-chunk DMA waits so the fused fp32 matmuls (whose
        # weight-load micro-op has a tight sync-wait budget) carry at most
        # a couple of semaphore waits each.
        nc.sync.dma_start(xt_tiles[0][:], xt_d.ap()[:, 0:XTW])
        tc.strict_bb_all_engine_barrier()
        for k in range(1, NXT):
            nc.sync.dma_start(xt_tiles[k][:], xt_d.ap()[:, k * XTW : (k + 1) * XTW])

        out_tiles = [
            out_pool.tile([NUM_NODES, OCW], f32, tag=f"out{k}", name=f"out_sb{k}")
            for k in range(NOC)
        ]

        for g in range(NG):
            xk, xoff = divmod(g * GROUP, XTW)
            rhs = xt_tiles[xk][:, xoff : xoff + GROUP]

            l1 = l1_pool.tile([128, 8 * GROUP], f32)
            for t in range(8):
                nc.tensor.matmul(
                    l1[:, t * GROUP : (t + 1) * GROUP],
                    w1_sb[:, t * 128 : (t + 1) * 128],
                    rhs,
                    start=True,
                    stop=True,
                )

            h = h_pool.tile([128, 8 * GROUP], f32r, tag="h")
            nc.scalar.activation(h[:], l1[:], mybir.ActivationFunctionType.Tanh)

            mus = mus_pool.tile([NUM_NODES, GROUP], f32, tag="mus")
            for t in range(8):
                nc.tensor.matmul(
                    mus[:],
                    w2t_sb[:, t * NUM_NODES : (t + 1) * NUM_NODES],
                    h[:, t * GROUP : (t + 1) * GROUP],
                    start=(t == 0),
                    stop=(t == 7),
                )

            ok, ooff = divmod(g * GROUP, OCW)
            nc.vector.tensor_scalar_add(
                out_tiles[ok][:, ooff : ooff + GROUP], mus[:], b2_sb
            )

            if (g + 1) % OUT_CHUNK == 0:
                k = g // OUT_CHUNK
                nc.sync.dma_start(
                    out_d.ap()[:, k * OCW : (k + 1) * OCW], out_tiles[k][:]
                )

    nc.finalize()
    return nc


def _get_nc():
    if "nc" not in _COMPILED:
        _COMPILED["nc"] = _build_nc()
    return _COMPILED["nc"]


def kernel(gt_labels, W1, b1, W2, b2, parent_idx):
    from concourse.bass_utils import run_bass_kernel_spmd

    gt_labels = np.asarray(gt_labels, np.float32)
    wpack = _build_weights(W1, b1, W2, b2, parent_idx)

    in_maps = []
    for c in range(NCORES):
        xc = gt_labels[c * BC : (c + 1) * BC]  # (16384, 64)
        xt = np.empty((NUM_NODES + 1, BC), np.float32)
        xt[:NUM_NODES] = xc.T
        xt[NUM_NODES] = 1.0
        in_maps.append({"xt": xt, "wpack": wpack})

    nc = _get_nc()
    trace = bool(int(os.environ.get("KERNEL_TRACE", "0")))
    res = run_bass_kernel_spmd(nc, in_maps, list(range(NCORES)), trace=trace)
    if trace and res.exec_time_ns is not None:
        print(f"HW exec time: {res.exec_time_ns} ns")
        _COMPILED["exec_time_ns"] = res.exec_time_ns

    mus = np.empty((BATCH, NUM_NODES), np.float32)
    for c in range(NCORES):
        mus[c * BC : (c + 1) * BC] = res.results[c]["out"].T
    mus = mus.reshape(BATCH, NUM_NODES, 1)
    logvars = np.zeros_like(mus)
    return mus, logvars



# revision 2
# speedup vs baseline: 1.5166x; 1.5166x over previous
"""Trainium2 Bass kernel for GroundTruthBasedPriorNetwork.

Per-node tiny MLP over a banded DAG, batched over 131072 samples:
    x[b, n, p]  = gt_labels[b, parent_idx[n, p]]          (N=64 nodes, P=8)
    h[b, n, :]  = tanh(W1[n] @ x[b, n, :] + b1[n])        (HID=16)
    mus[b, n]   = W2[n] . h[b, n, :] + b2[n]
    logvars     = zeros

Strategy: pure data parallel over 8 NeuronCores (batch split 8x16384).
The parent gather is folded on the host into a dense (65 x 1024) bf16
matrix W1aug (64 input rows + bias row) against a host-transposed input
XT (65 x 16384, bf16, with a ones row).  All matmuls run in bf16
(1 cycle/row on the PE, overlapped LDWEIGHTS; the old fp32r path
self-loads weights serially per matmul which doubled PE time).

Loop structure per 1536-column block (10 blocks + one 1024 tail):
  for t in 0..7:  # hidden block of 128 dims = nodes 8t..8t+8
    3x matmul (W1aug[:,128t:128t+128] stationary, 512-col slabs)
       -> l1 PSUM tile (128 x 1536), double-buffered (2x3 banks)
    one Tanh on ScalarE -> h_t (128 x 1536) bf16 in SBUF
  L2 of the PREVIOUS block is interleaved at t=2,4,6 (software
  pipelining) so ScalarE never idles: per 512-slab, 8 accumulating
  matmuls (W2T block (128,64) stationary) -> mus PSUM (64 x 512,
  2x1 bank), then one DVE tensor_scalar_add(b2) evacuates to SBUF.
ScalarE (tanh, ~132us) is the roofline engine; PE (~115us) and DVE
(~21us) hide underneath it.
"""

import os

import numpy as np

NUM_NODES = 64
MAX_P = 8
HID = 16
HFULL = NUM_NODES * HID  # 1024
BATCH = 131072
NCORES = 8
BC = BATCH // NCORES  # 16384 per core
SLAB = 512
CHUNK = 4096  # input/output DMA chunk width

_COMPILED = {}


def _bf16(a):
    import ml_dtypes

    return np.asarray(a, np.float32).astype(ml_dtypes.bfloat16)


def _build_weights(W1, b1, W2, b2, parent_idx):
    """Host-side preprocessing of the tiny per-node weights."""
    W1 = np.asarray(W1, np.float32)
    b1 = np.asarray(b1, np.float32)
    W2 = np.asarray(W2, np.float32)
    b2 = np.asarray(b2, np.float32)
    parent_idx = np.asarray(parent_idx)

    # W1_full[j, 16n+h] = sum_p [parent_idx[n,p]==j] * W1[n,h,p]
    w1_full = np.zeros((NUM_NODES, HFULL), np.float32)
    for n in range(NUM_NODES):
        for p in range(MAX_P):
            j = int(parent_idx[n, p])
            w1_full[j, 16 * n : 16 * n + 16] += W1[n, :, p]
    w1_aug = np.concatenate([w1_full, b1.reshape(1, HFULL)], axis=0)  # (65, 1024)

    # W2T[p, 64t+n] = W2[n, hf%16] where hf = 128t+p and n == hf//16, else 0
    w2t = np.zeros((128, 8 * NUM_NODES), np.float32)
    for t in range(8):
        for p in range(128):
            hf = 128 * t + p
            n = hf // HID
            w2t[p, NUM_NODES * t + n] = W2[n, hf % HID]

    wts = np.zeros((128, HFULL + 8 * NUM_NODES), np.float32)
    wts[: NUM_NODES + 1, :HFULL] = w1_aug
    wts[:, HFULL:] = w2t
    return _bf16(wts), np.ascontiguousarray(b2.reshape(NUM_NODES, 1))


def _block_widths():
    ws = []
    c = 0
    while c < BC:
        w = min(1536, BC - c)
        ws.append(w)
        c += w
    return ws


def _build_nc():
    import concourse.bacc as bacc
    import concourse.mybir as mybir
    import concourse.tile as tile
    from contextlib import ExitStack

    f32 = mybir.dt.float32
    bf16 = mybir.dt.bfloat16

    nc = bacc.Bacc("TRN2", target_bir_lowering=False, debug=False,
                   num_devices=NCORES)

    CW = HFULL + 8 * NUM_NODES  # 1536
    xt_d = nc.dram_tensor("xt", [NUM_NODES + 1, BC], bf16, kind="ExternalInput")
    wts_d = nc.dram_tensor("wts", [128, CW], bf16, kind="ExternalInput")
    b2_d = nc.dram_tensor("b2", [NUM_NODES, 1], f32, kind="ExternalInput")
    out_d = nc.dram_tensor("out", [NUM_NODES, BC], f32, kind="ExternalOutput")

    NXT = BC // CHUNK  # 4 input / output chunks

    widths = _block_widths()

    with tile.TileContext(nc) as tc, ExitStack() as ctx:
        consts = ctx.enter_context(tc.tile_pool(name="consts", bufs=1))
        xt_pool = ctx.enter_context(tc.tile_pool(name="xt", bufs=1))
        out_pool = ctx.enter_context(tc.tile_pool(name="out", bufs=1))
        h_pool = ctx.enter_context(tc.tile_pool(name="h", bufs=18))
        l1_pool = ctx.enter_context(tc.tile_pool(name="l1", bufs=2, space="PSUM"))
        mus_pool = ctx.enter_context(tc.tile_pool(name="mus", bufs=2, space="PSUM"))

        wts_sb = consts.tile([128, CW], bf16, tag="wts")
        b2_sb = consts.tile([NUM_NODES, 1], f32, tag="b2")
        dummy = consts.tile([128, 8], f32, tag="dummy")
        dummy2 = consts.tile([128, 8], bf16, tag="dummy2")
        nc.sync.dma_start(wts_sb[:], wts_d.ap())
        nc.sync.dma_start(b2_sb[:], b2_d.ap())
        w1_sb = wts_sb[: NUM_NODES + 1, :HFULL]
        w2_sb = wts_sb[:, HFULL:]

        # Pre-trigger the ACT tanh table load (~2.7us) while DMAs run.
        nc.vector.memset(dummy[:], 0.0)
        nc.scalar.activation(dummy2[:], dummy[:],
                             mybir.ActivationFunctionType.Tanh)

        xt_tiles = []
        for k in range(NXT):
            xt_sb = xt_pool.tile([NUM_NODES + 1, CHUNK], bf16, tag=f"xt{k}",
                                 name=f"xt_sb{k}")
            xt_tiles.append(xt_sb)
        nc.sync.dma_start(xt_tiles[0][:], xt_d.ap()[:, 0:CHUNK])
        tc.strict_bb_all_engine_barrier()
        for k in range(1, NXT):
            nc.sync.dma_start(xt_tiles[k][:], xt_d.ap()[:, k * CHUNK : (k + 1) * CHUNK])

        out_tiles = [
            out_pool.tile([NUM_NODES, CHUNK], f32, tag=f"out{k}", name=f"out_sb{k}")
            for k in range(NXT)
        ]
        out_done = [0] * NXT  # slabs evacuated per output chunk

        h_live = {}  # (block, t) -> h tile
        pending = []  # (block, col) L2 slab work items

        def emit_l2(item):
            b, c = item
            mus = mus_pool.tile([NUM_NODES, SLAB], f32, tag="mus")
            for t in range(8):
                h = h_live[(b, t)]
                hc = c - sum(widths[:b])
                nc.tensor.matmul(
                    mus[:],
                    w2_sb[:, NUM_NODES * t : NUM_NODES * (t + 1)],
                    h[:, hc : hc + SLAB],
                    start=(t == 0),
                    stop=(t == 7),
                )
            ok, oo = divmod(c, CHUNK)
            nc.vector.tensor_scalar_add(
                out_tiles[ok][:, oo : oo + SLAB], mus[:], b2_sb
            )
            out_done[ok] += 1
            if out_done[ok] == CHUNK // SLAB:
                nc.sync.dma_start(
                    out_d.ap()[:, ok * CHUNK : (ok + 1) * CHUNK], out_tiles[ok][:]
                )

        col0 = 0
        for b, W in enumerate(widths):
            for t in range(8):
                l1 = l1_pool.tile([128, W], f32, tag="l1")
                for s in range(W // SLAB):
                    c = col0 + s * SLAB
                    xk, xo = divmod(c, CHUNK)
                    nc.tensor.matmul(
                        l1[:, s * SLAB : (s + 1) * SLAB],
                        w1_sb[:, t * 128 : (t + 1) * 128],
                        xt_tiles[xk][:, xo : xo + SLAB],
                        start=True,
                        stop=True,
                    )
                h = h_pool.tile([128, W], bf16, tag="h")
                nc.scalar.activation(h[:], l1[:],
                                     mybir.ActivationFunctionType.Tanh)
                h_live[(b, t)] = h
                if t in (2, 4, 6) and pending:
                    emit_l2(pending.pop(0))
            for s in range(W // SLAB):
                pending.append((b, col0 + s * SLAB))
            col0 += W
        while pending:
            emit_l2(pending.pop(0))

    nc.finalize()
    return nc


def _get_nc():
    if "nc" not in _COMPILED:
        _COMPILED["nc"] = _build_nc()
    return _COMPILED["nc"]


def kernel(gt_labels, W1, b1, W2, b2, parent_idx):
    from concourse.bass_utils import run_bass_kernel_spmd

    gt_labels = np.asarray(gt_labels, np.float32)
    wts, b2c = _build_weights(W1, b1, W2, b2, parent_idx)

    in_maps = []
    for c in range(NCORES):
        xc = gt_labels[c * BC : (c + 1) * BC]  # (16384, 64)
        xt = np.empty((NUM_NODES + 1, BC), np.float32)
        xt[:NUM_NODES] = xc.T
        xt[NUM_NODES] = 1.0
        in_maps.append({"xt": _bf16(xt), "wts": wts, "b2": b2c})

    nc = _get_nc()
    trace = bool(int(os.environ.get("KERNEL_TRACE", "0")))
    res = run_bass_kernel_spmd(nc, in_maps, list(range(NCORES)), trace=trace)
    if trace and res.exec_time_ns is not None:
        print(f"HW exec time: {res.exec_time_ns} ns")
        _COMPILED["exec_time_ns"] = res.exec_time_ns

    mus = np.empty((BATCH, NUM_NODES), np.float32)
    for c in range(NCORES):
        mus[c * BC : (c + 1) * BC] = res.results[c]["out"].T
    mus = mus.reshape(BATCH, NUM_NODES, 1)
    logvars = np.zeros_like(mus)
    return mus, logvars


# revision 3
# speedup vs baseline: 1.8249x; 1.2033x over previous
"""Trainium2 Bass kernel for GroundTruthBasedPriorNetwork.

Per-node tiny MLP over a banded DAG, batched over 131072 samples:
    x[b, n, p]  = gt_labels[b, parent_idx[n, p]]          (N=64 nodes, P=8)
    h[b, n, :]  = tanh(W1[n] @ x[b, n, :] + b1[n])        (HID=16)
    mus[b, n]   = W2[n] . h[b, n, :] + b2[n]
    logvars     = zeros

Pure data parallel over 8 NeuronCores (batch split 8x16384).  ScalarE
(tanh over 16.8M elems/core, ~134us) is the roofline engine; the PE work
is packed with 32x32 array tiling so that even at the cold 1.2 GHz HAM
clock it hides underneath.

The banded DAG means hidden block t (128 dims = nodes 8t..8t+8) only
reads input rows [8t-8, 8t+7) plus a bias row: K=16.  Three blocks are
row-tiled into PE row-groups 0/32/64 and run concurrently (measured
~2.4-3x).  The host prepares band tensors xb_r (one per trio round r)
whose partition group j holds block t=3r+j's 15 input rows + ones row.

Layer 2 per block needs only an (128, 8) stationary (nodes 8t..8t+8);
three blocks are col-tiled into col-groups 0/32/64 writing partition
strips 32j..32j+8 of one PSUM bank, evacuated (plus b2) by a single
full-width DVE tensor_scalar_add per trio whose inactive lanes carry
junk that is never DMA'd out.

Per 512-col slab: 3 row-tiled L1 trios -> 2x(128,1536)+1x(128,1024)
PSUM tiles (2x3 banks, double-buffered), one Tanh each -> bf16 h,
then the previous slab's L2 (3 col-tiled trios -> mus bank x2) is
software-pipelined behind it.  Output rows leave in 3 node-strip
tensors reassembled on the host.
"""

import os

import numpy as np

NUM_NODES = 64
MAX_P = 8
HID = 16
HFULL = NUM_NODES * HID  # 1024
BATCH = 131072
NCORES = 8
BC = BATCH // NCORES  # 16384 per core
SLAB = 512
NSLAB = BC // SLAB  # 32
CHUNK = 4096  # input DMA chunk width
OC = 2048  # output DMA chunk width
TRIOS = ((0, 1, 2), (3, 4, 5), (6, 7))  # hidden-block trio per round

_COMPILED = {}


def _bf16(a):
    import ml_dtypes

    return np.asarray(a, np.float32).astype(ml_dtypes.bfloat16)


def _band_lo(t):
    return max(0, 8 * t - 8)


def _build_weights(W1, b1, W2, b2, parent_idx):
    """Host-side preprocessing of the tiny per-node weights."""
    W1 = np.asarray(W1, np.float32)
    b1 = np.asarray(b1, np.float32)
    W2 = np.asarray(W2, np.float32)
    b2 = np.asarray(b2, np.float32)
    parent_idx = np.asarray(parent_idx)

    # W1_full[j, 16n+h] = sum_p [parent_idx[n,p]==j] * W1[n,h,p]
    w1_full = np.zeros((NUM_NODES, HFULL), np.float32)
    for n in range(NUM_NODES):
        for p in range(MAX_P):
            j = int(parent_idx[n, p])
            w1_full[j, 16 * n : 16 * n + 16] += W1[n, :, p]

    # Row-tiled L1 stationaries: w1b[32j+i, 128r+c] = block t=3r+j's
    # weight for band row i (i=15 -> bias b1).
    w1b = np.zeros((128, 3 * 128), np.float32)
    for r, trio in enumerate(TRIOS):
        for j, t in enumerate(trio):
            lo = _band_lo(t)
            nrow = 8 * t + 7 - lo if t > 0 else 7
            w1b[32 * j : 32 * j + nrow, 128 * r : 128 * (r + 1)] = \
                w1_full[lo : lo + nrow, 128 * t : 128 * (t + 1)]
            w1b[32 * j + 15, 128 * r : 128 * (r + 1)] = b1.reshape(HFULL)[
                128 * t : 128 * (t + 1)
            ]

    # Col-tiled L2 stationaries: w2c[p, 8t+k] = W2[8t+k, (128t+p)%16]
    # where (128t+p)//16 == 8t+k, else 0.
    w2c = np.zeros((128, NUM_NODES), np.float32)
    for t in range(8):
        for p in range(128):
            hf = 128 * t + p
            n = hf // HID
            w2c[p, n] = W2[n, hf % HID]

    wts = np.zeros((128, 3 * 128 + NUM_NODES), np.float32)
    wts[:, : 3 * 128] = w1b
    wts[:, 3 * 128 :] = w2c

    # b2 packed into evacuation strip layout: col r, partition 32j+i.
    b2r = np.zeros((128, 3), np.float32)
    for r, trio in enumerate(TRIOS):
        for j, t in enumerate(trio):
            b2r[32 * j : 32 * j + 8, r] = b2[8 * t : 8 * t + 8]
    return _bf16(wts), np.ascontiguousarray(b2r)


def _build_bands(xc):
    """xc: (BC, 64) fp32 one core's batch. Returns 3 band tensors."""
    xt = xc.T  # (64, BC)
    outs = []
    for trio in TRIOS:
        xb = np.zeros((16 * len(trio), BC), np.float32)
        for j, t in enumerate(trio):
            lo = _band_lo(t)
            nrow = 8 * t + 7 - lo if t > 0 else 7
            xb[16 * j : 16 * j + nrow] = xt[lo : lo + nrow]
            xb[16 * j + 15] = 1.0
        outs.append(_bf16(xb))
    return outs


def _build_nc():
    import concourse.bacc as bacc
    import concourse.mybir as mybir
    import concourse.tile as tile
    from contextlib import ExitStack

    f32 = mybir.dt.float32
    bf16 = mybir.dt.bfloat16

    nc = bacc.Bacc("TRN2", target_bir_lowering=False, debug=False,
                   num_devices=NCORES)

    CW = 3 * 128 + NUM_NODES  # 448
    xb_d = [
        nc.dram_tensor(f"xb{r}", [16 * len(trio), BC], bf16,
                       kind="ExternalInput")
        for r, trio in enumerate(TRIOS)
    ]
    wts_d = nc.dram_tensor("wts", [128, CW], bf16, kind="ExternalInput")
    b2_d = nc.dram_tensor("b2", [128, 3], f32, kind="ExternalInput")
    out_d = [
        nc.dram_tensor(f"out{r}", [8 * len(trio), BC], f32,
                       kind="ExternalOutput")
        for r, trio in enumerate(TRIOS)
    ]

    with tile.TileContext(nc) as tc, ExitStack() as ctx:
        consts = ctx.enter_context(tc.tile_pool(name="consts", bufs=1))
        xb_pool = ctx.enter_context(tc.tile_pool(name="xb", bufs=1))
        out_pool = ctx.enter_context(tc.tile_pool(name="outp", bufs=2))
        h_pool = ctx.enter_context(tc.tile_pool(name="h", bufs=8))
        l1_pool = ctx.enter_context(tc.tile_pool(name="l1", bufs=2, space="PSUM"))
        mus_pool = ctx.enter_context(tc.tile_pool(name="mus", bufs=2, space="PSUM"))

        wts_sb = consts.tile([128, CW], bf16, tag="wts")
        b2_sb = consts.tile([128, 3], f32, tag="b2")
        dummy = consts.tile([128, 8], f32, tag="dummy")
        dummy2 = consts.tile([128, 8], bf16, tag="dummy2")
        nc.sync.dma_start(wts_sb[:], wts_d.ap())
        nc.sync.dma_start(b2_sb[:], b2_d.ap())
        w1b_sb = wts_sb[:, : 3 * 128]
        w2c_sb = wts_sb[:, 3 * 128 :]

        # Pre-trigger the ACT tanh table load (~2.7us) while DMAs run.
        nc.vector.memset(dummy[:], 0.0)
        nc.scalar.activation(dummy2[:], dummy[:],
                             mybir.ActivationFunctionType.Tanh)

        xb_sb = [
            xb_pool.tile([128, BC], bf16, tag=f"xb{r}", name=f"xb_sb{r}")
            for r in range(3)
        ]
        # First chunk of each band tensor, then barrier, then the rest.
        for r, trio in enumerate(TRIOS):
            for j in range(len(trio)):
                nc.sync.dma_start(
                    xb_sb[r][32 * j : 32 * j + 16, 0:CHUNK],
                    xb_d[r].ap()[16 * j : 16 * j + 16, 0:CHUNK],
                )
        tc.strict_bb_all_engine_barrier()
        for k in range(1, BC // CHUNK):
            for r, trio in enumerate(TRIOS):
                for j in range(len(trio)):
                    nc.sync.dma_start(
                        xb_sb[r][32 * j : 32 * j + 16, k * CHUNK : (k + 1) * CHUNK],
                        xb_d[r].ap()[16 * j : 16 * j + 16, k * CHUNK : (k + 1) * CHUNK],
                    )

        # Output chunk tiles: (128, OC) per round, strips 32j..32j+8 used.
        nout = BC // OC  # 8
        out_tiles = {}
        out_fill = {}

        def out_tile(r, k):
            if (r, k) not in out_tiles:
                out_tiles[(r, k)] = out_pool.tile(
                    [128, OC], f32, tag=f"or{r}", name=f"out_r{r}_k{k}"
                )
                out_fill[(r, k)] = 0
            return out_tiles[(r, k)]

        h_live = {}

        def emit_l2(s):
            for r, trio in enumerate(TRIOS):
                h = h_live.pop((s, r))
                mus = mus_pool.tile([128, SLAB], f32, tag="mus")
                for j, t in enumerate(trio):
                    nc.tensor.matmul(
                        mus[32 * j : 32 * j + 8, :],
                        w2c_sb[:, 8 * t : 8 * t + 8],
                        h[:, SLAB * j : SLAB * (j + 1)],
                        start=True,
                        stop=True,
                        tile_position=(0, 32 * j),
                    )
                k, oo = divmod(s * SLAB, OC)
                ot = out_tile(r, k)
                nc.vector.tensor_scalar_add(
                    ot[:, oo : oo + SLAB], mus[:], b2_sb[:, r : r + 1]
                )
                out_fill[(r, k)] += 1
                if out_fill[(r, k)] == OC // SLAB:
                    for j in range(len(trio)):
                        nc.sync.dma_start(
                            out_d[r].ap()[8 * j : 8 * j + 8, k * OC : (k + 1) * OC],
                            ot[32 * j : 32 * j + 8, :],
                        )
                    del out_tiles[(r, k)]

        for s in range(NSLAB):
            c = s * SLAB
            for r, trio in enumerate(TRIOS):
                W = SLAB * len(trio)
                l1 = l1_pool.tile([128, W], f32, tag="l1")
                for j in range(len(trio)):
                    nc.tensor.matmul(
                        l1[:, SLAB * j : SLAB * (j + 1)],
                        w1b_sb[32 * j : 32 * j + 16, 128 * r : 128 * (r + 1)],
                        xb_sb[r][32 * j : 32 * j + 16, c : c + SLAB],
                        start=True,
                        stop=True,
                        tile_position=(32 * j, 0),
                    )
                h = h_pool.tile([128, W], bf16, tag="h")
                nc.scalar.activation(h[:], l1[:],
                                     mybir.ActivationFunctionType.Tanh)
                h_live[(s, r)] = h
            if s > 0:
                emit_l2(s - 1)
        emit_l2(NSLAB - 1)

    nc.finalize()
    return nc


def _get_nc():
    if "nc" not in _COMPILED:
        _COMPILED["nc"] = _build_nc()
    return _COMPILED["nc"]


def kernel(gt_labels, W1, b1, W2, b2, parent_idx):
    from concourse.bass_utils import run_bass_kernel_spmd

    gt_labels = np.asarray(gt_labels, np.float32)
    wts, b2r = _build_weights(W1, b1, W2, b2, parent_idx)

    in_maps = []
    for c in range(NCORES):
        xb = _build_bands(gt_labels[c * BC : (c + 1) * BC])
        in_maps.append(
            {"xb0": xb[0], "xb1": xb[1], "xb2": xb[2], "wts": wts, "b2": b2r}
        )

    nc = _get_nc()
    trace = bool(int(os.environ.get("KERNEL_TRACE", "0")))
    res = run_bass_kernel_spmd(nc, in_maps, list(range(NCORES)), trace=trace)
    if trace and res.exec_time_ns is not None:
        print(f"HW exec time: {res.exec_time_ns} ns")
        _COMPILED["exec_time_ns"] = res.exec_time_ns

    mus = np.empty((BATCH, NUM_NODES), np.float32)
    for c in range(NCORES):
        full = np.concatenate(
            [res.results[c][f"out{r}"] for r in range(3)], axis=0
        )  # (64, BC) rows = nodes in order
        mus[c * BC : (c + 1) * BC] = full.T
    mus = mus.reshape(BATCH, NUM_NODES, 1)
    logvars = np.zeros_like(mus)
    return mus, logvars


# revision 5
# speedup vs baseline: 1.9416x; 1.0639x over previous
"""Trainium2 Bass kernel for GroundTruthBasedPriorNetwork.

Per-node tiny MLP over a banded DAG, batched over 131072 samples:
    x[b, n, p]  = gt_labels[b, parent_idx[n, p]]          (N=64 nodes, P=8)
    h[b, n, :]  = tanh(W1[n] @ x[b, n, :] + b1[n])        (HID=16)
    mus[b, n]   = W2[n] . h[b, n, :] + b2[n]
    logvars     = zeros

Pure data parallel over 8 NeuronCores (batch split 8x16384).  ScalarE
(tanh over 16.8M elems/core) is the roofline engine; everything else is
shaped to hide beneath it, assuming the cold 1.2 GHz PE clock (HAM
never warms on an ACT-bound kernel).

The banded DAG means hidden block t (128 dims = nodes 8t..8t+8) only
reads input rows [8t-8, 8t+7) plus a bias row: K=16.  Four blocks are
row-tiled into PE row-groups 0/32/64/96 and run concurrently; the host
prepares band tensors xbA/xbB (quads t=0-3 / t=4-7) whose partition
group j holds block t's 15 input rows + a ones row.  One quad fills a
full (128, 2048) PSUM tile (4 banks); one Tanh per quad (64 total,
the minimum the 8-bank PSUM allows with double buffering).

Layer 2 per block needs only a (128, 8) stationary; four blocks are
col-tiled into col-groups 0/32/64/96, writing partition strips
32j..32j+8 of bank 0 of the SAME l1 quad-tile the Tanh just consumed
(no separate mus pool - PSUM is exactly 2x4 banks).  One full-width
DVE tensor_scalar_add(b2) evacuates the strips (inactive lanes carry
junk, never DMA'd).  The chain tanhA -> L2A -> evacA -> refill-A fits
inside tanhB's duration, so ScalarE stays ~100% busy.

Outputs leave as two bf16 node-strip tensors (nodes 0-31 / 32-63),
reassembled and cast on the host.
"""

import os

import numpy as np

NUM_NODES = 64
MAX_P = 8
HID = 16
HFULL = NUM_NODES * HID  # 1024
BATCH = 131072
NCORES = 8
BC = BATCH // NCORES  # 16384 per core
SLAB = 512
NSLAB = BC // SLAB  # 32
QW = 4 * SLAB  # 2048: quad tile width
OC = 2048  # output DMA chunk width
QUADS = ((0, 1, 2, 3), (4, 5, 6, 7))

_COMPILED = {}


def _bf16(a):
    import ml_dtypes

    return np.asarray(a, np.float32).astype(ml_dtypes.bfloat16)


def _band_lo(t):
    return max(0, 8 * t - 8)


def _build_weights(W1, b1, W2, b2, parent_idx):
    """Host-side preprocessing of the tiny per-node weights."""
    W1 = np.asarray(W1, np.float32)
    b1 = np.asarray(b1, np.float32)
    W2 = np.asarray(W2, np.float32)
    b2 = np.asarray(b2, np.float32)
    parent_idx = np.asarray(parent_idx)

    # W1_full[j, 16n+h] = sum_p [parent_idx[n,p]==j] * W1[n,h,p]
    w1_full = np.zeros((NUM_NODES, HFULL), np.float32)
    for n in range(NUM_NODES):
        for p in range(MAX_P):
            j = int(parent_idx[n, p])
            w1_full[j, 16 * n : 16 * n + 16] += W1[n, :, p]

    # Row-tiled L1 stationaries: w1b[32j+i, 128q+c] = block t=4q+j's
    # weight for band row i (i=15 -> bias b1).
    w1b = np.zeros((128, 2 * 128), np.float32)
    for q, quad in enumerate(QUADS):
        for j, t in enumerate(quad):
            lo = _band_lo(t)
            nrow = 8 * t + 7 - lo if t > 0 else 7
            w1b[32 * j : 32 * j + nrow, 128 * q : 128 * (q + 1)] = \
                w1_full[lo : lo + nrow, 128 * t : 128 * (t + 1)]
            w1b[32 * j + 15, 128 * q : 128 * (q + 1)] = b1.reshape(HFULL)[
                128 * t : 128 * (t + 1)
            ]

    # Col-tiled L2 stationaries: w2c[p, 8t+k] = W2[8t+k, (128t+p)%16]
    # where (128t+p)//16 == 8t+k, else 0.
    w2c = np.zeros((128, NUM_NODES), np.float32)
    for t in range(8):
        for p in range(128):
            hf = 128 * t + p
            n = hf // HID
            w2c[p, n] = W2[n, hf % HID]

    wts = np.zeros((128, 2 * 128 + NUM_NODES), np.float32)
    wts[:, : 2 * 128] = w1b
    wts[:, 2 * 128 :] = w2c

    # b2 packed into evacuation strip layout: col q, partition 32j+i.
    b2r = np.zeros((128, 2), np.float32)
    for q, quad in enumerate(QUADS):
        for j, t in enumerate(quad):
            b2r[32 * j : 32 * j + 8, q] = b2[8 * t : 8 * t + 8]
    return _bf16(wts), np.ascontiguousarray(b2r)


def _build_bands(xc):
    """xc: (BC, 64) fp32 one core's batch. Returns 2 quad band tensors."""
    xt = xc.T  # (64, BC)
    outs = []
    for quad in QUADS:
        xb = np.zeros((64, BC), np.float32)
        for j, t in enumerate(quad):
            lo = _band_lo(t)
            nrow = 8 * t + 7 - lo if t > 0 else 7
            xb[16 * j : 16 * j + nrow] = xt[lo : lo + nrow]
            xb[16 * j + 15] = 1.0
        outs.append(_bf16(xb))
    return outs


def _build_nc():
    import concourse.bacc as bacc
    import concourse.mybir as mybir
    import concourse.tile as tile
    from contextlib import ExitStack

    f32 = mybir.dt.float32
    bf16 = mybir.dt.bfloat16

    nc = bacc.Bacc("TRN2", target_bir_lowering=False, debug=False,
                   num_devices=NCORES)

    CW = 2 * 128 + NUM_NODES  # 320
    xb_d = [
        nc.dram_tensor(f"xb{q}", [64, BC], bf16, kind="ExternalInput")
        for q in range(2)
    ]
    wts_d = nc.dram_tensor("wts", [128, CW], bf16, kind="ExternalInput")
    b2_d = nc.dram_tensor("b2", [128, 2], f32, kind="ExternalInput")
    out_d = [
        nc.dram_tensor(f"out{q}", [32, BC], bf16, kind="ExternalOutput")
        for q in range(2)
    ]

    # Input DMA chunking: small first chunk so compute starts early.
    ICHUNKS = [(0, 1024), (1024, 3072), (4096, 4096), (8192, 4096),
               (12288, 4096)]

    with tile.TileContext(nc) as tc, ExitStack() as ctx:
        consts = ctx.enter_context(tc.tile_pool(name="consts", bufs=1))
        xb_pool = ctx.enter_context(tc.tile_pool(name="xb", bufs=1))
        out_pool = ctx.enter_context(tc.tile_pool(name="outp", bufs=2))
        h_pool = ctx.enter_context(tc.tile_pool(name="h", bufs=6))
        l1_pool = ctx.enter_context(tc.tile_pool(name="l1", bufs=2, space="PSUM"))

        wts_sb = consts.tile([128, CW], bf16, tag="wts")
        b2_sb = consts.tile([128, 2], f32, tag="b2")
        dummy = consts.tile([128, 8], f32, tag="dummy")
        dummy2 = consts.tile([128, 8], bf16, tag="dummy2")
        nc.sync.dma_start(wts_sb[:], wts_d.ap())
        nc.sync.dma_start(b2_sb[:], b2_d.ap())
        w1b_sb = wts_sb[:, : 2 * 128]
        w2c_sb = wts_sb[:, 2 * 128 :]

        # Pre-trigger the ACT tanh table load (~2.7us) while DMAs run.
        nc.vector.memset(dummy[:], 0.0)
        nc.scalar.activation(dummy2[:], dummy[:],
                             mybir.ActivationFunctionType.Tanh)

        xb_sb = [
            xb_pool.tile([128, BC], bf16, tag=f"xb{q}", name=f"xb_sb{q}")
            for q in range(2)
        ]
        for q in range(2):
            c0, w = ICHUNKS[0]
            for j in range(4):
                nc.sync.dma_start(
                    xb_sb[q][32 * j : 32 * j + 16, c0 : c0 + w],
                    xb_d[q].ap()[16 * j : 16 * j + 16, c0 : c0 + w],
                )
        tc.strict_bb_all_engine_barrier()
        for c0, w in ICHUNKS[1:]:
            for q in range(2):
                for j in range(4):
                    nc.sync.dma_start(
                        xb_sb[q][32 * j : 32 * j + 16, c0 : c0 + w],
                        xb_d[q].ap()[16 * j : 16 * j + 16, c0 : c0 + w],
                    )

        # Output chunk tiles: (128, OC) bf16 per quad, strips used only.
        out_tiles = {}
        out_fill = {}

        def out_tile(q, k):
            if (q, k) not in out_tiles:
                out_tiles[(q, k)] = out_pool.tile(
                    [128, OC], bf16, tag=f"oq{q}", name=f"out_q{q}_k{k}"
                )
                out_fill[(q, k)] = 0
            return out_tiles[(q, k)]

        h_live = {}
        l1_live = {}

        def emit_l2(s, q):
            h = h_live.pop((s, q))
            l1 = l1_live.pop((s, q))
            quad = QUADS[q]
            for j, t in enumerate(quad):
                nc.tensor.matmul(
                    l1[32 * j : 32 * j + 8, 0:SLAB],
                    w2c_sb[:, 8 * t : 8 * t + 8],
                    h[:, SLAB * j : SLAB * (j + 1)],
                    start=True,
                    stop=True,
                    tile_position=(0, 32 * j),
                    skip_group_check=True,
                )
            k, oo = divmod(s * SLAB, OC)
            ot = out_tile(q, k)
            nc.vector.tensor_scalar_add(
                ot[:, oo : oo + SLAB], l1[:, 0:SLAB], b2_sb[:, q : q + 1]
            )
            out_fill[(q, k)] += 1
            if out_fill[(q, k)] == OC // SLAB:
                for j in range(4):
                    nc.sync.dma_start(
                        out_d[q].ap()[8 * j : 8 * j + 8, k * OC : (k + 1) * OC],
                        ot[32 * j : 32 * j + 8, :],
                    )
                del out_tiles[(q, k)]

        for s in range(NSLAB):
            c = s * SLAB
            for q in range(2):
                if s > 0:
                    emit_l2(s - 1, q)
                l1 = l1_pool.tile([128, QW], f32, tag="l1")
                # Bank 0 (j=0) is refilled last: the previous slab's mus
                # strips live there and the DVE evacuation overlaps the
                # j=1..3 matmuls.
                for j in (1, 2, 3, 0):
                    nc.tensor.matmul(
                        l1[:, SLAB * j : SLAB * (j + 1)],
                        w1b_sb[32 * j : 32 * j + 16, 128 * q : 128 * (q + 1)],
                        xb_sb[q][32 * j : 32 * j + 16, c : c + SLAB],
                        start=True,
                        stop=True,
                        tile_position=(32 * j, 0),
                    )
                h = h_pool.tile([128, QW], bf16, tag="h")
                nc.scalar.activation(h[:], l1[:],
                                     mybir.ActivationFunctionType.Tanh)
                h_live[(s, q)] = h
                l1_live[(s, q)] = l1
        for q in range(2):
            emit_l2(NSLAB - 1, q)

    nc.finalize()
    return nc


def _get_nc():
    if "nc" not in _COMPILED:
        _COMPILED["nc"] = _build_nc()
    return _COMPILED["nc"]


def kernel(gt_labels, W1, b1, W2, b2, parent_idx):
    from concourse.bass_utils import run_bass_kernel_spmd

    gt_labels = np.asarray(gt_labels, np.float32)
    wts, b2r = _build_weights(W1, b1, W2, b2, parent_idx)

    in_maps = []
    for c in range(NCORES):
        xb = _build_bands(gt_labels[c * BC : (c + 1) * BC])
        in_maps.append({"xb0": xb[0], "xb1": xb[1], "wts": wts, "b2": b2r})

    nc = _get_nc()
    trace = bool(int(os.environ.get("KERNEL_TRACE", "0")))
    res = run_bass_kernel_spmd(nc, in_maps, list(range(NCORES)), trace=trace)
    if trace and res.exec_time_ns is not None:
        print(f"HW exec time: {res.exec_time_ns} ns")
        _COMPILED["exec_time_ns"] = res.exec_time_ns

    mus = np.empty((BATCH, NUM_NODES), np.float32)
    for c in range(NCORES):
        full = np.concatenate(
            [np.asarray(res.results[c][f"out{q}"], np.float32) for q in range(2)],
            axis=0,
        )  # (64, BC) rows = nodes in order
        mus[c * BC : (c + 1) * BC] = full.T
    mus = mus.reshape(BATCH, NUM_NODES, 1)
    logvars = np.zeros_like(mus)
    return mus, logvars


# revision 7
# speedup vs baseline: 2.4632x; 1.2687x over previous
"""Trainium2 Bass kernel for GroundTruthBasedPriorNetwork.

Per-node tiny MLP over a banded DAG, batched over 131072 samples:
    x[b, n, p]  = gt_labels[b, parent_idx[n, p]]          (N=64 nodes, P=8)
    h[b, n, :]  = tanh(W1[n] @ x[b, n, :] + b1[n])        (HID=16)
    mus[b, n]   = W2[n] . h[b, n, :] + b2[n]
    logvars     = zeros

Pure data parallel over 8 NeuronCores (batch split 8x16384).  ScalarE
(tanh over 16.8M elems/core; saturated ACT pays ~790 cycles of access
setup per instruction, so 64 x (128,2048) tiles ~= 151us) is the
roofline engine; everything else is shaped to hide beneath it at the
cold 1.2 GHz PE clock.

The banded DAG means hidden block t (128 dims = nodes 8t..8t+8) only
reads input rows [8t-8, 8t+7) plus a bias row: K=16.  Four blocks are
row-tiled into PE row-groups 0/32/64/96 and run concurrently; the host
prepares band panels xb0/xb1 (quads t=0-3 / t=4-7, 128 partitions with
a ones row per 32-group) so one quad fills a (128, 2048) PSUM tile.
PSUM is exactly 2 quad tiles (2x4 banks, double-buffered).

Layer 2 per block needs only a (128, 8) stationary; four blocks are
col-tiled into col-groups 0/32/64/96, writing partition strips
32j..32j+8 of bank 0 of the SAME l1 quad-tile its Tanh just consumed.
One full-width DVE tensor_scalar_add(b2) evacuates the strips
(inactive lanes carry junk that the host discards).  Bank 0 of each
quad is refilled last (j order 1,2,3,0) so the evacuation overlaps the
other three matmuls.

All PE/ACT/DVE instructions are chained with order-only dependencies
(add_dep_helper) pinning the software-pipelined emission order; the
Tile scheduler's cost model otherwise reorders the FIFO and causes
head-of-line blocking on the evacuate->refill edge.

Outputs leave as two (128, BC) bf16 panels (junk lanes included);
the host extracts node strips and casts.
"""

import os

import numpy as np

NUM_NODES = 64
MAX_P = 8
HID = 16
HFULL = NUM_NODES * HID  # 1024
BATCH = 131072
NCORES = 8
BC = BATCH // NCORES  # 16384 per core
SLAB = 512
NSLAB = BC // SLAB  # 32
QW = 4 * SLAB  # 2048: quad tile width
OC = 2048  # output DMA chunk width
QUADS = ((0, 1, 2, 3), (4, 5, 6, 7))
ICHUNKS = [(0, 1024), (1024, 3072), (4096, 4096), (8192, 4096),
           (12288, 4096)]

_COMPILED = {}


def _bf16(a):
    import ml_dtypes

    return np.asarray(a, np.float32).astype(ml_dtypes.bfloat16)


def _band_lo(t):
    return max(0, 8 * t - 8)


def _build_weights(W1, b1, W2, b2, parent_idx):
    """Host-side preprocessing of the tiny per-node weights."""
    W1 = np.asarray(W1, np.float32)
    b1 = np.asarray(b1, np.float32)
    W2 = np.asarray(W2, np.float32)
    b2 = np.asarray(b2, np.float32)
    parent_idx = np.asarray(parent_idx)

    # W1_full[j, 16n+h] = sum_p [parent_idx[n,p]==j] * W1[n,h,p]
    w1_full = np.zeros((NUM_NODES, HFULL), np.float32)
    for n in range(NUM_NODES):
        for p in range(MAX_P):
            j = int(parent_idx[n, p])
            w1_full[j, 16 * n : 16 * n + 16] += W1[n, :, p]

    # Row-tiled L1 stationaries: w1b[32j+i, 128q+c] = block t=4q+j's
    # weight for band row i (i=15 -> bias b1).
    w1b = np.zeros((128, 2 * 128), np.float32)
    for q, quad in enumerate(QUADS):
        for j, t in enumerate(quad):
            lo = _band_lo(t)
            nrow = 8 * t + 7 - lo if t > 0 else 7
            w1b[32 * j : 32 * j + nrow, 128 * q : 128 * (q + 1)] = \
                w1_full[lo : lo + nrow, 128 * t : 128 * (t + 1)]
            w1b[32 * j + 15, 128 * q : 128 * (q + 1)] = b1.reshape(HFULL)[
                128 * t : 128 * (t + 1)
            ]

    # Col-tiled L2 stationaries: w2c[p, 8t+k] = W2[8t+k, (128t+p)%16]
    # where (128t+p)//16 == 8t+k, else 0.
    w2c = np.zeros((128, NUM_NODES), np.float32)
    for t in range(8):
        for p in range(128):
            hf = 128 * t + p
            n = hf // HID
            w2c[p, n] = W2[n, hf % HID]

    wts = np.zeros((128, 2 * 128 + NUM_NODES), np.float32)
    wts[:, : 2 * 128] = w1b
    wts[:, 2 * 128 :] = w2c

    # b2 packed into evacuation strip layout: col q, partition 32j+i.
    b2r = np.zeros((128, 2), np.float32)
    for q, quad in enumerate(QUADS):
        for j, t in enumerate(quad):
            b2r[32 * j : 32 * j + 8, q] = b2[8 * t : 8 * t + 8]
    return _bf16(wts), np.ascontiguousarray(b2r)


def _build_bands(xc):
    """xc: (BC, 64) fp32 one core's batch. Returns 2 quad band panels."""
    xt = xc.T  # (64, BC)
    outs = []
    for quad in QUADS:
        xb = np.zeros((128, BC), np.float32)
        for j, t in enumerate(quad):
            lo = _band_lo(t)
            nrow = 8 * t + 7 - lo if t > 0 else 7
            xb[32 * j : 32 * j + nrow] = xt[lo : lo + nrow]
            xb[32 * j + 15] = 1.0
        outs.append(_bf16(xb))
    return outs


def _build_nc():
    import concourse.bacc as bacc
    import concourse.mybir as mybir
    import concourse.tile as tile
    from concourse.tile import add_dep_helper
    from contextlib import ExitStack

    f32 = mybir.dt.float32
    bf16 = mybir.dt.bfloat16

    nc = bacc.Bacc("TRN2", target_bir_lowering=False, debug=False,
                   num_devices=NCORES)

    CW = 2 * 128 + NUM_NODES  # 320
    xb_d = [
        nc.dram_tensor(f"xb{q}", [128, BC], bf16, kind="ExternalInput")
        for q in range(2)
    ]
    wts_d = nc.dram_tensor("wts", [128, CW], bf16, kind="ExternalInput")
    b2_d = nc.dram_tensor("b2", [128, 2], f32, kind="ExternalInput")
    out_d = [
        nc.dram_tensor(f"out{q}", [128, BC], bf16, kind="ExternalOutput")
        for q in range(2)
    ]

    last = {}  # per-engine previous instruction, for order pinning

    def pin(key, bi):
        if key in last:
            add_dep_helper(bi.ins, last[key].ins, sync=False,
                           reason="pipeline order")
        last[key] = bi
        return bi

    with tile.TileContext(nc) as tc, ExitStack() as ctx:
        consts = ctx.enter_context(tc.tile_pool(name="consts", bufs=1))
        xb_pool = ctx.enter_context(tc.tile_pool(name="xb", bufs=1))
        out_pool = ctx.enter_context(tc.tile_pool(name="outp", bufs=2))
        h_pool = ctx.enter_context(tc.tile_pool(name="h", bufs=6))
        l1_pool = ctx.enter_context(tc.tile_pool(name="l1", bufs=2, space="PSUM"))

        wts_sb = consts.tile([128, CW], bf16, tag="wts")
        b2_sb = consts.tile([128, 2], f32, tag="b2")
        dummy = consts.tile([128, 8], f32, tag="dummy")
        dummy2 = consts.tile([128, 8], bf16, tag="dummy2")
        nc.sync.dma_start(wts_sb[:], wts_d.ap())
        nc.sync.dma_start(b2_sb[:], b2_d.ap())
        w1b_sb = wts_sb[:, : 2 * 128]
        w2c_sb = wts_sb[:, 2 * 128 :]

        # Pre-trigger the ACT tanh table load (~2.7us) while DMAs run.
        nc.vector.memset(dummy[:], 0.0)
        nc.scalar.activation(dummy2[:], dummy[:],
                             mybir.ActivationFunctionType.Tanh)

        xb_sb = [
            xb_pool.tile([128, BC], bf16, tag=f"xb{q}", name=f"xb_sb{q}")
            for q in range(2)
        ]
        c0, w = ICHUNKS[0]
        for q in range(2):
            nc.sync.dma_start(xb_sb[q][:, c0 : c0 + w],
                              xb_d[q].ap()[:, c0 : c0 + w])
        tc.strict_bb_all_engine_barrier()
        for c0, w in ICHUNKS[1:]:
            for q in range(2):
                nc.sync.dma_start(xb_sb[q][:, c0 : c0 + w],
                                  xb_d[q].ap()[:, c0 : c0 + w])

        out_tiles = {}
        out_fill = {}

        def out_tile(q, k):
            if (q, k) not in out_tiles:
                out_tiles[(q, k)] = out_pool.tile(
                    [128, OC], bf16, tag=f"oq{q}", name=f"out_q{q}_k{k}"
                )
                out_fill[(q, k)] = 0
            return out_tiles[(q, k)]

        h_live = {}
        l1_live = {}

        def emit_l2(s, q):
            h = h_live.pop((s, q))
            l1 = l1_live.pop((s, q))
            for j, t in enumerate(QUADS[q]):
                pin("pe", nc.tensor.matmul(
                    l1[32 * j : 32 * j + 8, 0:SLAB],
                    w2c_sb[:, 8 * t : 8 * t + 8],
                    h[:, SLAB * j : SLAB * (j + 1)],
                    start=True,
                    stop=True,
                    tile_position=(0, 32 * j),
                    skip_group_check=True,
                ))
            k, oo = divmod(s * SLAB, OC)
            ot = out_tile(q, k)
            pin("dve", nc.vector.tensor_scalar_add(
                ot[:, oo : oo + SLAB], l1[:, 0:SLAB], b2_sb[:, q : q + 1]
            ))
            out_fill[(q, k)] += 1
            if out_fill[(q, k)] == OC // SLAB:
                nc.sync.dma_start(
                    out_d[q].ap()[:, k * OC : (k + 1) * OC], ot[:]
                )
                del out_tiles[(q, k)]

        for s in range(NSLAB):
            c = s * SLAB
            for q in range(2):
                if s > 0:
                    emit_l2(s - 1, q)
                l1 = l1_pool.tile([128, QW], f32, tag="l1")
                # Bank 0 (j=0) is refilled last: the previous slab's mus
                # strips live there and the DVE evacuation overlaps the
                # j=1..3 matmuls.
                for j in (1, 2, 3, 0):
                    pin("pe", nc.tensor.matmul(
                        l1[:, SLAB * j : SLAB * (j + 1)],
                        w1b_sb[32 * j : 32 * j + 16, 128 * q : 128 * (q + 1)],
                        xb_sb[q][32 * j : 32 * j + 16, c : c + SLAB],
                        start=True,
                        stop=True,
                        tile_position=(32 * j, 0),
                    ))
                h = h_pool.tile([128, QW], bf16, tag="h")
                pin("act", nc.scalar.activation(
                    h[:], l1[:], mybir.ActivationFunctionType.Tanh))
                h_live[(s, q)] = h
                l1_live[(s, q)] = l1
        for q in range(2):
            emit_l2(NSLAB - 1, q)

    nc.finalize()
    return nc


def _get_nc():
    if "nc" not in _COMPILED:
        _COMPILED["nc"] = _build_nc()
    return _COMPILED["nc"]


def kernel(gt_labels, W1, b1, W2, b2, parent_idx):
    from concourse.bass_utils import run_bass_kernel_spmd

    gt_labels = np.asarray(gt_labels, np.float32)
    wts, b2r = _build_weights(W1, b1, W2, b2, parent_idx)

    in_maps = []
    for c in range(NCORES):
        xb = _build_bands(gt_labels[c * BC : (c + 1) * BC])
        in_maps.append({"xb0": xb[0], "xb1": xb[1], "wts": wts, "b2": b2r})

    nc = _get_nc()
    trace = bool(int(os.environ.get("KERNEL_TRACE", "0")))
    res = run_bass_kernel_spmd(nc, in_maps, list(range(NCORES)), trace=trace)
    if trace and res.exec_time_ns is not None:
        print(f"HW exec time: {res.exec_time_ns} ns")
        _COMPILED["exec_time_ns"] = res.exec_time_ns

    mus = np.empty((BATCH, NUM_NODES), np.float32)
    for c in range(NCORES):
        rows = []
        for q in range(2):
            panel = np.asarray(res.results[c][f"out{q}"], np.float32)
            for j in range(4):
                rows.append(panel[32 * j : 32 * j + 8])  # nodes 32q+8j..+8
        mus[c * BC : (c + 1) * BC] = np.concatenate(rows, axis=0).T
    mus = mus.reshape(BATCH, NUM_NODES, 1)
    logvars = np.zeros_like(mus)
    return mus, logvars
